# revision 21
# baseline (speedup 1.0000x reference)
"""Trainium2 Bass kernel for NoisyPositionEmbedder (embedding_lookup).

Computation (A=1024 atoms, T=256 tokens):
  cl  += LN(a2t @ s_trunk) @ W_s            [A, 128]
  plm += LN(z_atom) @ W_z                   [A, A, 16],  z_atom[a,b,:] = zij[tok[a],tok[b],:]
  ql  += rl @ W_r                           [A, 128]

Strategy: 8-way shard over atoms, with atoms SORTED BY TOKEN so each
core's 128 atom rows touch only a <=W=64 token window of zij. Per core:
precompute Y[t,u,:] = LN(zij[t,u,:]) @ W_z' for the window only, then the
plm update is two one-hot matmul gathers (exact) + the plm add, which is
folded into the same PSUM accumulation via an identity matmul.
LN gain/bias are folded into the projection: W' = diag(g)@W, b' = b@W.
"""

import numpy as np
from contextlib import ExitStack

import concourse.bass as bass
import concourse.bacc as bacc
import concourse.mybir as mybir
import concourse.tile as tile
from concourse.bass import ds
from concourse.bass_utils import run_bass_kernel_spmd
from concourse.masks import make_identity

F32 = mybir.dt.float32
F32R = mybir.dt.float32r

# problem shapes (hardcoded per contract)
A = 1024          # atoms
T = 256           # tokens
CS = 384          # c_s
CZ = 128          # c_z
CA = 128          # c_atom
CP = 16           # c_pair
M = 8             # cores
S = A // M        # 128 atom rows per core
W = 36            # padded distinct-token count per core (compacted)
EPS = 1e-5

NPAIRS = W * T            # (t,u) pairs per core in stage 1
NT = NPAIRS // 128        # 128-pair tiles
NQ = 6                    # 4-tile groups per super-batch
NSB = NT // (4 * NQ)      # super-batches of 24 tiles (3072 pairs)
BC = 512                  # b-chunk for stage B
NBC = A // BC             # 4 chunks

_PROG_CACHE = {}


def _alt(i, *engines):
    return engines[i % len(engines)]


def _copy(i, nc, out, in_):
    """psum->sbuf copy, alternating ScalarE / VectorE by index."""
    if i % 2 == 0:
        nc.scalar.copy(out=out, in_=in_)
    else:
        nc.vector.tensor_copy(out=out, in_=in_)


def build_program():
    nc = bacc.Bacc("TRN2")

    # ---------------- I/O ----------------
    zij_win = nc.dram_tensor("zij_win", [W, T, CZ], F32, kind="ExternalInput")
    a2tT = nc.dram_tensor("a2tT", [T, A], F32R, kind="ExternalInput")
    a2tT_win = nc.dram_tensor("a2tT_win", [W, S], F32R, kind="ExternalInput")
    plm_slab = nc.dram_tensor("plm_slab", [S, A, CP], F32, kind="ExternalInput")
    cl_slab = nc.dram_tensor("cl_slab", [S, CA], F32, kind="ExternalInput")
    ql_slab = nc.dram_tensor("ql_slab", [S, CA], F32, kind="ExternalInput")
    rlT_slab = nc.dram_tensor("rlT_slab", [3, S], F32, kind="ExternalInput")
    strunk_win = nc.dram_tensor("strunk_win", [W, CS], F32R, kind="ExternalInput")
    wzp = nc.dram_tensor("wzp", [CZ, CP], F32R, kind="ExternalInput")
    bz = nc.dram_tensor("bz", [CP, 1], F32, kind="ExternalInput")
    wsp = nc.dram_tensor("wsp", [CS, CA], F32, kind="ExternalInput")
    bs = nc.dram_tensor("bs", [1, CA], F32, kind="ExternalInput")
    wr = nc.dram_tensor("wr", [3, CA], F32, kind="ExternalInput")

    cl_out = nc.dram_tensor("cl_out", [S, CA], F32, kind="ExternalOutput")
    ql_out = nc.dram_tensor("ql_out", [S, CA], F32, kind="ExternalOutput")
    plm_out = nc.dram_tensor("plm_out", [S, A, CP], F32, kind="ExternalOutput")

    Identity = mybir.ActivationFunctionType.Identity
    Sqrt = mybir.ActivationFunctionType.Sqrt
    SUB = mybir.AluOpType.subtract
    ADD = mybir.AluOpType.add
    MULT = mybir.AluOpType.mult

    with TileCtx(nc) as (tc, ctx):
        const = ctx.enter_context(tc.tile_pool(name="const", bufs=1))
        dram = ctx.enter_context(tc.tile_pool(name="dram", bufs=1, space="DRAM"))
        p_zt = ctx.enter_context(tc.tile_pool(name="p_zt", bufs=14))
        p_st = ctx.enter_context(tc.tile_pool(name="p_st", bufs=3))
        p_sm = ctx.enter_context(tc.tile_pool(name="p_sm", bufs=2))
        p_zn = ctx.enter_context(tc.tile_pool(name="p_zn", bufs=6))
        p_tps = ctx.enter_context(tc.tile_pool(name="p_tps", bufs=2, space="PSUM"))
        p_zns = ctx.enter_context(tc.tile_pool(name="p_zns", bufs=3))
        p_ytp = ctx.enter_context(tc.tile_pool(name="p_ytp", bufs=1, space="PSUM"))
        p_yts = ctx.enter_context(tc.tile_pool(name="p_yts", bufs=2))
        p_yw = ctx.enter_context(tc.tile_pool(name="p_yw", bufs=1))
        p_aps = ctx.enter_context(tc.tile_pool(name="p_aps", bufs=2, space="PSUM"))
        p_clps = ctx.enter_context(tc.tile_pool(name="p_clps", bufs=1, space="PSUM"))
        p_yh = ctx.enter_context(tc.tile_pool(name="p_yh", bufs=1))
        p_plm = ctx.enter_context(tc.tile_pool(name="p_plm", bufs=2))
        p_bps = ctx.enter_context(tc.tile_pool(name="p_bps", bufs=2, space="PSUM"))
        p_cl = ctx.enter_context(tc.tile_pool(name="p_cl", bufs=2))

        # ------------- constants -------------
        ident = const.tile([128, 128], F32)
        make_identity(nc, ident)
        identr = const.tile([128, 128], F32R)
        nc.vector.tensor_copy(out=identr, in_=ident)
        wzp_s = const.tile([CZ, CP], F32R)
        nc.sync.dma_start(out=wzp_s, in_=wzp[:, :])
        bz_s = const.tile([CP, 1], F32)
        nc.sync.dma_start(out=bz_s, in_=bz[:, :])
        a2tT_s = const.tile([128, 2, A], F32R)
        nc.sync.dma_start(out=a2tT_s, in_=a2tT.rearrange("(c p) b -> p c b", p=128))
        a2tT_win_s = const.tile([W, S], F32R)
        nc.sync.dma_start(out=a2tT_win_s, in_=a2tT_win[:, :])
        strunk_s = const.tile([W, CS], F32R)
        nc.sync.dma_start(out=strunk_s, in_=strunk_win[:, :])
        wsp_s = const.tile([128, 3, CA], F32)
        nc.sync.dma_start(out=wsp_s, in_=wsp.rearrange("(k p) n -> p k n", p=128))
        bs_s = const.tile([1, CA], F32)
        nc.sync.dma_start(out=bs_s, in_=bs[:, :])
        wr_s = const.tile([3, CA], F32)
        nc.sync.dma_start(out=wr_s, in_=wr[:, :])
        rlT_s = const.tile([3, S], F32)
        nc.sync.dma_start(out=rlT_s, in_=rlT_slab[:, :])
        ones_s = const.tile([1, S], F32)
        nc.vector.memset(ones_s, 1.0)
        epsz = const.tile([128, 1], F32)   # CZ * EPS for folded var
        nc.vector.memset(epsz, float(CZ) * EPS)
        eps1 = const.tile([128, 1], F32)
        nc.vector.memset(eps1, EPS)

        ytd = dram.tile([CP, NPAIRS], F32R)   # Y^T scratch in DRAM
        y_win = p_yw.tile([W, CP, T], F32R)   # Y in [t, (k,u)] layout

        # ------------- stage 1: Y^T = (LN(zij_win) @ W_z')^T -------------
        zf = zij_win.rearrange("t u c -> (t u) c")
        for sb in range(NSB):
            zts = []
            for q in range(NQ):
                zt = p_zt.tile([128, 4, CZ], F32, tag="zt")
                nc.sync.dma_start(
                    out=zt,
                    in_=zf[ds(sb * NQ * 512 + q * 512, 512), :].rearrange(
                        "(j p) c -> p j c", p=128
                    ),
                )
                zts.append(zt)
            st16 = p_st.tile([128, NQ, 4, 6], F32)
            for q in range(NQ):
                for j in range(4):
                    nc.vector.bn_stats(out=st16[:, q, j, :], in_=zts[q][:, j, :])
            # batched mean/var math over the whole super-batch:
            #   rstd = 1/sqrt((C+D)/128 + ((A-B)/2)^2 + eps); nmr = -0.5*(A+B)*rstd
            sA = st16[:, :, :, 1:2]
            sB = st16[:, :, :, 4:5]
            sC = st16[:, :, :, 2:3]
            sD = st16[:, :, :, 5:6]
            t_ab = p_sm.tile([128, NQ, 4, 1], F32, tag="t_ab")
            t_amb = p_sm.tile([128, NQ, 4, 1], F32, tag="t_amb")
            sq = p_sm.tile([128, NQ, 4, 1], F32, tag="sq")
            t_cd = p_sm.tile([128, NQ, 4, 1], F32, tag="t_cd")
            t3 = p_sm.tile([128, NQ, 4, 1], F32, tag="t3")
            sd = p_sm.tile([128, NQ, 4, 1], F32, tag="sd")
            rstd = p_sm.tile([128, NQ, 4, 1], F32, tag="rstd")
            nmr = p_sm.tile([128, NQ, 4, 1], F32, tag="nmr")
            nc.vector.tensor_tensor(out=t_ab, in0=sA, in1=sB, op=ADD)
            nc.vector.tensor_tensor(out=t_amb, in0=sA, in1=sB, op=SUB)
            nc.vector.tensor_tensor(out=sq, in0=t_amb, in1=t_amb, op=MULT)
            nc.vector.tensor_tensor(out=t_cd, in0=sC, in1=sD, op=ADD)
            nc.vector.scalar_tensor_tensor(
                out=t3, in0=sq, scalar=float(CZ) / 4.0, in1=t_cd, op0=MULT, op1=ADD
            )
            # sd = sqrt(t3/128 + eps)
            nc.scalar.activation(out=sd, in_=t3, func=Sqrt, bias=eps1,
                                 scale=1.0 / float(CZ))
            nc.vector.reciprocal(out=rstd, in_=sd)
            nc.vector.scalar_tensor_tensor(
                out=nmr, in0=t_ab, scalar=-0.5, in1=rstd, op0=MULT, op1=MULT
            )

            for half in range(2):
                yt_s = p_yts.tile([CP, 1536], F32R, name="yt_s")
                for qh in range(3):
                    q = half * 3 + qh
                    zn = p_zn.tile([128, 4, CZ], F32R, tag="zn")
                    for j in range(4):
                        r_ap = rstd[:, q, j, 0:1]
                        n_ap = nmr[:, q, j, 0:1]
                        if j % 2 == 0:
                            nc.gpsimd.tensor_scalar(
                                out=zn[:, j, :], in0=zts[q][:, j, :],
                                scalar1=r_ap, scalar2=n_ap, op0=MULT, op1=ADD,
                            )
                        else:
                            nc.scalar.activation(
                                out=zn[:, j, :], in_=zts[q][:, j, :],
                                func=Identity, bias=n_ap, scale=r_ap,
                            )
                    znt_ps = p_tps.tile([128, 512], F32R)
                    for j in range(4):
                        nc.tensor.transpose(znt_ps[:, ds(j * 128, 128)], zn[:, j, :], identr)
                    znt_s = p_zns.tile([128, 512], F32R)
                    _copy(qh, nc, znt_s, znt_ps)
                    yt_ps = p_ytp.tile([CP, 512], F32)
                    nc.tensor.matmul(yt_ps, lhsT=wzp_s, rhs=znt_s, start=True, stop=True)
                    nc.scalar.activation(
                        out=yt_s[:, ds(qh * 512, 512)], in_=yt_ps,
                        func=Identity, bias=bz_s, scale=1.0,
                    )
                nc.sync.dma_start(
                    out=ytd[:, ds(sb * 3072 + half * 1536, 1536)], in_=yt_s
                )

            # relayout this super-batch's 12 t-rows back as [t, (k,u)]
            nc.sync.dma_start(
                out=y_win[ds(sb * 12, 12), :, :],
                in_=ytd.rearrange("k (t u) -> t k u", t=W)[ds(sb * 12, 12), :, :],
            )

        # ------------- stage A: YhT[u, (k,a)] = Y_win[:,k,uc*128:...]^T @ a2tT_win
        yh = [
            p_yh.tile([128, CP, 128], F32R, tag=f"yh{uc}", name=f"yh{uc}")
            for uc in range(2)
        ]
        for uc in range(2):
            for kq in range(4):
                ps = p_aps.tile([128, 4, 128], F32)
                for k4 in range(4):
                    k = kq * 4 + k4
                    nc.tensor.matmul(
                        ps[:, k4, :], lhsT=y_win[:, k, ds(uc * 128, 128)],
                        rhs=a2tT_win_s, start=True, stop=True,
                    )
                nc.scalar.copy(out=yh[uc][:, ds(kq * 4, 4), :], in_=ps)

        # ------------- stage B: plm rows -------------
        for bc in range(NBC):
            plm_t = p_plm.tile([S, BC, CP], F32)
            nc.gpsimd.dma_start(out=plm_t, in_=plm_slab[:, ds(bc * BC, BC), :])
            for k in range(CP):
                pr = p_bps.tile([128, BC], F32)
                nc.tensor.matmul(
                    pr, lhsT=yh[0][:, k, :],
                    rhs=a2tT_s[:, 0, ds(bc * BC, BC)],
                    start=True, stop=False,
                )
                nc.tensor.matmul(
                    pr, lhsT=yh[1][:, k, :],
                    rhs=a2tT_s[:, 1, ds(bc * BC, BC)],
                    start=False, stop=True,
                )
                # plm_t[:, :, k] = pr + plm_t[:, :, k]  (fused add on DVE)
                nc.vector.scalar_tensor_tensor(
                    out=plm_t[:, :, k], in0=pr, scalar=1.0,
                    in1=plm_t[:, :, k], op0=MULT, op1=ADD,
                )
            nc.sync.dma_start(
                out=plm_out[:, ds(bc * BC, BC // 2), :], in_=plm_t[:, : BC // 2, :]
            )
            nc.gpsimd.dma_start(
                out=plm_out[:, ds(bc * BC + BC // 2, BC // 2), :],
                in_=plm_t[:, BC // 2 :, :],
            )

        # ------------- cl -------------
        sa_ps = p_clps.tile([128, CS], F32, tag="clps")
        nc.tensor.matmul(sa_ps, lhsT=a2tT_win_s, rhs=strunk_s, start=True, stop=True)
        stc = p_cl.tile([128, 6], F32, tag="stc")
        nc.vector.bn_stats(out=stc, in_=sa_ps)
        mvc = p_cl.tile([128, 2], F32, tag="mvc")
        nc.vector.bn_aggr(out=mvc, in_=stc)
        sdc = p_cl.tile([128, 1], F32, tag="sdc")
        nc.scalar.activation(out=sdc, in_=mvc[:, 1:2], func=Sqrt, bias=eps1, scale=1.0)
        rsc = p_cl.tile([128, 1], F32, tag="rsc")
        nc.vector.reciprocal(out=rsc, in_=sdc)
        sa_n = p_cl.tile([128, CS], F32, tag="sa_n")
        nc.vector.tensor_scalar(
            out=sa_n, in0=sa_ps, scalar1=mvc[:, 0:1], scalar2=rsc, op0=SUB, op1=MULT
        )
        saT_ps = p_clps.tile([128, CS], F32, tag="clps", name="saT_ps")
        for j in range(3):
            nc.tensor.transpose(saT_ps[:, ds(j * 128, 128)], sa_n[:, ds(j * 128, 128)], ident)
        saT_s = p_cl.tile([128, 3, 128], F32, tag="saT_s")
        nc.scalar.copy(out=saT_s, in_=saT_ps.rearrange("p (j f) -> p j f", j=3))
        cl_ps = p_clps.tile([128, CA], F32, tag="clps", name="cl_ps")
        for j in range(3):
            nc.tensor.matmul(
                cl_ps, lhsT=saT_s[:, j, :], rhs=wsp_s[:, j, :],
                start=(j == 0), stop=False,
            )
        nc.tensor.matmul(cl_ps, lhsT=ones_s, rhs=bs_s, start=False, stop=True)
        cl_sl = p_cl.tile([S, CA], F32, tag="cl_sl")
        nc.sync.dma_start(out=cl_sl, in_=cl_slab[:, :])
        cl_o = p_cl.tile([S, CA], F32, tag="cl_o")
        nc.vector.tensor_tensor(out=cl_o, in0=cl_ps, in1=cl_sl, op=ADD)
        nc.sync.dma_start(out=cl_out[:, :], in_=cl_o)

        # ------------- ql -------------
        ql_ps = p_clps.tile([128, CA], F32, tag="clps", name="ql_ps")
        nc.tensor.matmul(ql_ps, lhsT=rlT_s, rhs=wr_s, start=True, stop=True)
        ql_sl = p_cl.tile([S, CA], F32, tag="ql_sl")
        nc.sync.dma_start(out=ql_sl, in_=ql_slab[:, :])
        ql_o = p_cl.tile([S, CA], F32, tag="ql_o")
        nc.vector.tensor_tensor(out=ql_o, in0=ql_ps, in1=ql_sl, op=ADD)
        nc.sync.dma_start(out=ql_out[:, :], in_=ql_o)

    nc.finalize()
    return nc


class TileCtx:
    """TileContext + ExitStack in one `with`."""

    def __init__(self, nc):
        self.nc = nc

    def __enter__(self):
        self.ctx = ExitStack()
        self.tc = self.ctx.enter_context(tile.TileContext(self.nc))
        return self.tc, self.ctx

    def __exit__(self, *exc):
        return self.ctx.__exit__(*exc)


def get_program():
    if "nc" not in _PROG_CACHE:
        _PROG_CACHE["nc"] = build_program()
    return _PROG_CACHE["nc"]


def _prep_inputs(inputs):
    f = lambda k: np.ascontiguousarray(np.asarray(inputs[k], dtype=np.float32))
    a2t = f("atom_to_token_index")
    cl = f("cl")
    plm = f("plm")
    ql = f("ql")
    s_trunk = f("s_trunk")
    zij = f("zij")
    rl = f("rl")
    ln_s_g = f("ln_s_g")
    ln_s_b = f("ln_s_b")
    W_s = f("W_s")
    ln_z_g = f("ln_z_g")
    ln_z_b = f("ln_z_b")
    W_z = f("W_z")
    W_r = f("W_r")

    tok = np.argmax(a2t, axis=1)
    perm = np.argsort(tok, kind="stable")

    wzp = np.ascontiguousarray(ln_z_g[:, None] * W_z)
    bz = np.ascontiguousarray((ln_z_b @ W_z)[:, None])          # [CP, 1]
    wsp = np.ascontiguousarray(ln_s_g[:, None] * W_s)
    bs = np.ascontiguousarray((ln_s_b @ W_s)[None, :])          # [1, CA]
    a2tT = np.ascontiguousarray(a2t.T)

    in_maps = []
    slabs = []
    for d in range(M):
        sl = perm[d * S : (d + 1) * S]
        ts_ = tok[sl]
        uts = np.unique(ts_)
        assert len(uts) <= W, f"{len(uts)} distinct tokens exceeds W={W}"
        idx = np.concatenate([uts, np.full(W - len(uts), uts[-1], uts.dtype)])
        pos = np.searchsorted(uts, ts_)
        a2tT_win = np.zeros((W, S), np.float32)
        a2tT_win[pos, np.arange(S)] = 1.0
        in_maps.append(
            {
                "zij_win": np.ascontiguousarray(zij[idx]),
                "a2tT": a2tT,
                "a2tT_win": a2tT_win,
                "plm_slab": np.ascontiguousarray(plm[sl]),
                "cl_slab": np.ascontiguousarray(cl[sl]),
                "ql_slab": np.ascontiguousarray(ql[sl]),
                "rlT_slab": np.ascontiguousarray(rl[sl].T),
                "strunk_win": np.ascontiguousarray(s_trunk[idx]),
                "wzp": wzp,
                "bz": bz,
                "wsp": wsp,
                "bs": bs,
                "wr": W_r,
            }
        )
        slabs.append(sl)
    return in_maps, slabs


def kernel(**inputs):
    in_maps, slabs = _prep_inputs(inputs)
    nc = get_program()
    res = run_bass_kernel_spmd(nc, in_maps, core_ids=list(range(M)))
    outs = res.results

    cl_full = np.empty((A, CA), np.float32)
    ql_full = np.empty((A, CA), np.float32)
    plm_full = np.empty((A, A, CP), np.float32)
    for d in range(M):
        sl = slabs[d]
        cl_full[sl] = outs[d]["cl_out"]
        ql_full[sl] = outs[d]["ql_out"]
        plm_full[sl] = outs[d]["plm_out"].reshape(S, A, CP)
    return (cl_full, plm_full, ql_full)


# revision 22
# speedup vs baseline: 1.2100x; 1.2100x over previous
"""Trainium2 Bass kernel for NoisyPositionEmbedder (embedding_lookup).

Computation (A=1024 atoms, T=256 tokens):
  cl  += LN(a2t @ s_trunk) @ W_s            [A, 128]
  plm += LN(z_atom) @ W_z                   [A, A, 16],  z_atom[a,b,:] = zij[tok[a],tok[b],:]
  ql  += rl @ W_r                           [A, 128]

Strategy: 8-way shard over atoms, with atoms SORTED BY TOKEN so each
core's 128 atom rows touch only a <=W=64 token window of zij. Per core:
precompute Y[t,u,:] = LN(zij[t,u,:]) @ W_z' for the window only, then the
plm update is two one-hot matmul gathers (exact) + the plm add, which is
folded into the same PSUM accumulation via an identity matmul.
LN gain/bias are folded into the projection: W' = diag(g)@W, b' = b@W.
"""

import numpy as np
from contextlib import ExitStack

import concourse.bass as bass
import concourse.bacc as bacc
import concourse.mybir as mybir
import concourse.tile as tile
from concourse.bass import ds
from concourse.bass_utils import run_bass_kernel_spmd
from concourse.masks import make_identity

F32 = mybir.dt.float32
F32R = mybir.dt.float32r

# problem shapes (hardcoded per contract)
A = 1024          # atoms
T = 256           # tokens
CS = 384          # c_s
CZ = 128          # c_z
CA = 128          # c_atom
CP = 16           # c_pair
M = 8             # cores
S = A // M        # 128 atom rows per core
W = 36            # padded distinct-token count per core (compacted)
EPS = 1e-5

NPAIRS = W * T            # (t,u) pairs per core in stage 1
NT = NPAIRS // 128        # 128-pair tiles
NQ = 6                    # 4-tile groups per super-batch
NSB = NT // (4 * NQ)      # super-batches of 24 tiles (3072 pairs)
BC = 512                  # b-chunk for stage B
NBC = A // BC             # 4 chunks

_PROG_CACHE = {}


def _alt(i, *engines):
    return engines[i % len(engines)]


def _copy(i, nc, out, in_):
    """psum->sbuf copy, alternating ScalarE / VectorE by index."""
    if i % 2 == 0:
        nc.scalar.copy(out=out, in_=in_)
    else:
        nc.vector.tensor_copy(out=out, in_=in_)


def build_program():
    nc = bacc.Bacc("TRN2")

    # ---------------- I/O ----------------
    zij_win = nc.dram_tensor("zij_win", [W, T, CZ], F32, kind="ExternalInput")
    a2tT = nc.dram_tensor("a2tT", [T, A], F32R, kind="ExternalInput")
    a2tT_win = nc.dram_tensor("a2tT_win", [W, S], F32R, kind="ExternalInput")
    plm_slab = nc.dram_tensor("plm_slab", [S, A, CP], F32, kind="ExternalInput")
    cl_slab = nc.dram_tensor("cl_slab", [S, CA], F32, kind="ExternalInput")
    ql_slab = nc.dram_tensor("ql_slab", [S, CA], F32, kind="ExternalInput")
    rlT_slab = nc.dram_tensor("rlT_slab", [3, S], F32, kind="ExternalInput")
    strunk_win = nc.dram_tensor("strunk_win", [W, CS], F32R, kind="ExternalInput")
    wzp = nc.dram_tensor("wzp", [CZ, CP], F32R, kind="ExternalInput")
    bz = nc.dram_tensor("bz", [CP, 1], F32, kind="ExternalInput")
    wsp = nc.dram_tensor("wsp", [CS, CA], F32, kind="ExternalInput")
    bs = nc.dram_tensor("bs", [1, CA], F32, kind="ExternalInput")
    wr = nc.dram_tensor("wr", [3, CA], F32, kind="ExternalInput")

    cl_out = nc.dram_tensor("cl_out", [S, CA], F32, kind="ExternalOutput")
    ql_out = nc.dram_tensor("ql_out", [S, CA], F32, kind="ExternalOutput")
    plm_out = nc.dram_tensor("plm_out", [S, A, CP], F32, kind="ExternalOutput")

    Identity = mybir.ActivationFunctionType.Identity
    Sqrt = mybir.ActivationFunctionType.Sqrt
    SUB = mybir.AluOpType.subtract
    ADD = mybir.AluOpType.add
    MULT = mybir.AluOpType.mult

    with TileCtx(nc) as (tc, ctx):
        const = ctx.enter_context(tc.tile_pool(name="const", bufs=1))
        dram = ctx.enter_context(tc.tile_pool(name="dram", bufs=1, space="DRAM"))
        p_zt = ctx.enter_context(tc.tile_pool(name="p_zt", bufs=14))
        p_st = ctx.enter_context(tc.tile_pool(name="p_st", bufs=3))
        p_sm = ctx.enter_context(tc.tile_pool(name="p_sm", bufs=2))
        p_zn = ctx.enter_context(tc.tile_pool(name="p_zn", bufs=6))
        p_tps = ctx.enter_context(tc.tile_pool(name="p_tps", bufs=2, space="PSUM"))
        p_zns = ctx.enter_context(tc.tile_pool(name="p_zns", bufs=3))
        p_ytp = ctx.enter_context(tc.tile_pool(name="p_ytp", bufs=1, space="PSUM"))
        p_yts = ctx.enter_context(tc.tile_pool(name="p_yts", bufs=2))
        p_yw = ctx.enter_context(tc.tile_pool(name="p_yw", bufs=1))
        p_aps = ctx.enter_context(tc.tile_pool(name="p_aps", bufs=2, space="PSUM"))
        p_clps = ctx.enter_context(tc.tile_pool(name="p_clps", bufs=1, space="PSUM"))
        p_yh = ctx.enter_context(tc.tile_pool(name="p_yh", bufs=1))
        p_plm = ctx.enter_context(tc.tile_pool(name="p_plm", bufs=2))
        p_bps = ctx.enter_context(tc.tile_pool(name="p_bps", bufs=2, space="PSUM"))
        p_cl = ctx.enter_context(tc.tile_pool(name="p_cl", bufs=2))

        # ------------- constants -------------
        ident = const.tile([128, 128], F32)
        make_identity(nc, ident)
        identr = const.tile([128, 128], F32R)
        nc.vector.tensor_copy(out=identr, in_=ident)
        wzp_s = const.tile([CZ, CP], F32R)
        nc.sync.dma_start(out=wzp_s, in_=wzp[:, :])
        bz_s = const.tile([CP, 1], F32)
        nc.sync.dma_start(out=bz_s, in_=bz[:, :])
        a2tT_s = const.tile([128, 2, A], F32R)
        nc.sync.dma_start(out=a2tT_s, in_=a2tT.rearrange("(c p) b -> p c b", p=128))
        a2tT_win_s = const.tile([W, S], F32R)
        nc.sync.dma_start(out=a2tT_win_s, in_=a2tT_win[:, :])
        strunk_s = const.tile([W, CS], F32R)
        nc.sync.dma_start(out=strunk_s, in_=strunk_win[:, :])
        wsp_s = const.tile([128, 3, CA], F32)
        nc.sync.dma_start(out=wsp_s, in_=wsp.rearrange("(k p) n -> p k n", p=128))
        bs_s = const.tile([1, CA], F32)
        nc.sync.dma_start(out=bs_s, in_=bs[:, :])
        wr_s = const.tile([3, CA], F32)
        nc.sync.dma_start(out=wr_s, in_=wr[:, :])
        rlT_s = const.tile([3, S], F32)
        nc.sync.dma_start(out=rlT_s, in_=rlT_slab[:, :])
        ones_s = const.tile([1, S], F32)
        nc.vector.memset(ones_s, 1.0)
        epsz = const.tile([128, 1], F32)   # CZ * EPS for folded var
        nc.vector.memset(epsz, float(CZ) * EPS)
        eps1 = const.tile([128, 1], F32)
        nc.vector.memset(eps1, EPS)

        ytd = dram.tile([CP, NPAIRS], F32R)   # Y^T scratch in DRAM
        y_win = p_yw.tile([W, CP, T], F32R)   # Y in [t, (k,u)] layout

        # ------------- stage 1: Y^T = (LN(zij_win) @ W_z')^T -------------
        zf = zij_win.rearrange("t u c -> (t u) c")
        for sb in range(NSB):
            zts = []
            for q in range(NQ):
                zt = p_zt.tile([128, 4, CZ], F32, tag="zt")
                nc.sync.dma_start(
                    out=zt,
                    in_=zf[ds(sb * NQ * 512 + q * 512, 512), :].rearrange(
                        "(j p) c -> p j c", p=128
                    ),
                )
                zts.append(zt)
            st16 = p_st.tile([128, NQ, 4, 6], F32)
            for q in range(NQ):
                for j in range(4):
                    nc.vector.bn_stats(out=st16[:, q, j, :], in_=zts[q][:, j, :])
            # batched mean/var math over the whole super-batch:
            #   rstd = 1/sqrt((C+D)/128 + ((A-B)/2)^2 + eps); nmr = -0.5*(A+B)*rstd
            sA = st16[:, :, :, 1:2]
            sB = st16[:, :, :, 4:5]
            sC = st16[:, :, :, 2:3]
            sD = st16[:, :, :, 5:6]
            t_ab = p_sm.tile([128, NQ, 4, 1], F32, tag="t_ab")
            t_amb = p_sm.tile([128, NQ, 4, 1], F32, tag="t_amb")
            sq = p_sm.tile([128, NQ, 4, 1], F32, tag="sq")
            t_cd = p_sm.tile([128, NQ, 4, 1], F32, tag="t_cd")
            t3 = p_sm.tile([128, NQ, 4, 1], F32, tag="t3")
            sd = p_sm.tile([128, NQ, 4, 1], F32, tag="sd")
            rstd = p_sm.tile([128, NQ, 4, 1], F32, tag="rstd")
            nmr = p_sm.tile([128, NQ, 4, 1], F32, tag="nmr")
            nc.vector.tensor_tensor(out=t_ab, in0=sA, in1=sB, op=ADD)
            nc.vector.tensor_tensor(out=t_amb, in0=sA, in1=sB, op=SUB)
            nc.vector.tensor_tensor(out=sq, in0=t_amb, in1=t_amb, op=MULT)
            nc.vector.tensor_tensor(out=t_cd, in0=sC, in1=sD, op=ADD)
            nc.vector.scalar_tensor_tensor(
                out=t3, in0=sq, scalar=float(CZ) / 4.0, in1=t_cd, op0=MULT, op1=ADD
            )
            # sd = sqrt(t3/128 + eps)
            nc.scalar.activation(out=sd, in_=t3, func=Sqrt, bias=eps1,
                                 scale=1.0 / float(CZ))
            nc.vector.reciprocal(out=rstd, in_=sd)
            nc.vector.scalar_tensor_tensor(
                out=nmr, in0=t_ab, scalar=-0.5, in1=rstd, op0=MULT, op1=MULT
            )

            for half in range(2):
                yt_s = p_yts.tile([CP, 1536], F32R, name="yt_s")
                for qh in range(3):
                    q = half * 3 + qh
                    zn = p_zn.tile([128, 4, CZ], F32R, tag="zn")
                    for j in range(4):
                        r_ap = rstd[:, q, j, 0:1]
                        n_ap = nmr[:, q, j, 0:1]
                        if j % 2 == 0:
                            nc.gpsimd.tensor_scalar(
                                out=zn[:, j, :], in0=zts[q][:, j, :],
                                scalar1=r_ap, scalar2=n_ap, op0=MULT, op1=ADD,
                            )
                        else:
                            nc.scalar.activation(
                                out=zn[:, j, :], in_=zts[q][:, j, :],
                                func=Identity, bias=n_ap, scale=r_ap,
                            )
                    znt_ps = p_tps.tile([128, 512], F32R)
                    for j in range(4):
                        nc.tensor.transpose(znt_ps[:, ds(j * 128, 128)], zn[:, j, :], identr)
                    znt_s = p_zns.tile([128, 512], F32R)
                    _copy(qh, nc, znt_s, znt_ps)
                    yt_ps = p_ytp.tile([CP, 512], F32)
                    nc.tensor.matmul(yt_ps, lhsT=wzp_s, rhs=znt_s, start=True, stop=True)
                    nc.scalar.activation(
                        out=yt_s[:, ds(qh * 512, 512)], in_=yt_ps,
                        func=Identity, bias=bz_s, scale=1.0,
                    )
                nc.sync.dma_start(
                    out=ytd[:, ds(sb * 3072 + half * 1536, 1536)], in_=yt_s
                )

            # relayout this super-batch's 12 t-rows back as [t, (k,u)]
            nc.sync.dma_start(
                out=y_win[ds(sb * 12, 12), :, :],
                in_=ytd.rearrange("k (t u) -> t k u", t=W)[ds(sb * 12, 12), :, :],
            )

        # ------------- stage A: YhT[u, (k,a)] = Y_win[:,k,uc*128:...]^T @ a2tT_win
        yh = [
            p_yh.tile([128, CP, 128], F32R, tag=f"yh{uc}", name=f"yh{uc}")
            for uc in range(2)
        ]
        for uc in range(2):
            for kq in range(4):
                ps = p_aps.tile([128, 4, 128], F32)
                for k4 in range(4):
                    k = kq * 4 + k4
                    nc.tensor.matmul(
                        ps[:, k4, :], lhsT=y_win[:, k, ds(uc * 128, 128)],
                        rhs=a2tT_win_s, start=True, stop=True,
                    )
                nc.scalar.copy(out=yh[uc][:, ds(kq * 4, 4), :], in_=ps)

        # ------------- stage B: plm rows -------------
        for bc in range(NBC):
            plm_t = p_plm.tile([S, BC, CP], F32)
            nc.sync.dma_start(out=plm_t, in_=plm_slab[:, ds(bc * BC, BC), :])
            for k in range(CP):
                pr = p_bps.tile([128, BC], F32)
                nc.tensor.matmul(
                    pr, lhsT=yh[0][:, k, :],
                    rhs=a2tT_s[:, 0, ds(bc * BC, BC)],
                    start=True, stop=False,
                )
                nc.tensor.matmul(
                    pr, lhsT=yh[1][:, k, :],
                    rhs=a2tT_s[:, 1, ds(bc * BC, BC)],
                    start=False, stop=True,
                )
                # plm_t[:, :, k] = pr + plm_t[:, :, k]  (fused add on DVE)
                nc.vector.scalar_tensor_tensor(
                    out=plm_t[:, :, k], in0=pr, scalar=1.0,
                    in1=plm_t[:, :, k], op0=MULT, op1=ADD,
                )
            nc.sync.dma_start(
                out=plm_out[:, ds(bc * BC, BC // 2), :], in_=plm_t[:, : BC // 2, :]
            )
            nc.gpsimd.dma_start(
                out=plm_out[:, ds(bc * BC + BC // 2, BC // 2), :],
                in_=plm_t[:, BC // 2 :, :],
            )

        # ------------- cl -------------
        sa_ps = p_clps.tile([128, CS], F32, tag="clps")
        nc.tensor.matmul(sa_ps, lhsT=a2tT_win_s, rhs=strunk_s, start=True, stop=True)
        stc = p_cl.tile([128, 6], F32, tag="stc")
        nc.vector.bn_stats(out=stc, in_=sa_ps)
        mvc = p_cl.tile([128, 2], F32, tag="mvc")
        nc.vector.bn_aggr(out=mvc, in_=stc)
        sdc = p_cl.tile([128, 1], F32, tag="sdc")
        nc.scalar.activation(out=sdc, in_=mvc[:, 1:2], func=Sqrt, bias=eps1, scale=1.0)
        rsc = p_cl.tile([128, 1], F32, tag="rsc")
        nc.vector.reciprocal(out=rsc, in_=sdc)
        sa_n = p_cl.tile([128, CS], F32, tag="sa_n")
        nc.vector.tensor_scalar(
            out=sa_n, in0=sa_ps, scalar1=mvc[:, 0:1], scalar2=rsc, op0=SUB, op1=MULT
        )
        saT_ps = p_clps.tile([128, CS], F32, tag="clps", name="saT_ps")
        for j in range(3):
            nc.tensor.transpose(saT_ps[:, ds(j * 128, 128)], sa_n[:, ds(j * 128, 128)], ident)
        saT_s = p_cl.tile([128, 3, 128], F32, tag="saT_s")
        nc.scalar.copy(out=saT_s, in_=saT_ps.rearrange("p (j f) -> p j f", j=3))
        cl_ps = p_clps.tile([128, CA], F32, tag="clps", name="cl_ps")
        for j in range(3):
            nc.tensor.matmul(
                cl_ps, lhsT=saT_s[:, j, :], rhs=wsp_s[:, j, :],
                start=(j == 0), stop=False,
            )
        nc.tensor.matmul(cl_ps, lhsT=ones_s, rhs=bs_s, start=False, stop=True)
        cl_sl = p_cl.tile([S, CA], F32, tag="cl_sl")
        nc.sync.dma_start(out=cl_sl, in_=cl_slab[:, :])
        cl_o = p_cl.tile([S, CA], F32, tag="cl_o")
        nc.vector.tensor_tensor(out=cl_o, in0=cl_ps, in1=cl_sl, op=ADD)
        nc.sync.dma_start(out=cl_out[:, :], in_=cl_o)

        # ------------- ql -------------
        ql_ps = p_clps.tile([128, CA], F32, tag="clps", name="ql_ps")
        nc.tensor.matmul(ql_ps, lhsT=rlT_s, rhs=wr_s, start=True, stop=True)
        ql_sl = p_cl.tile([S, CA], F32, tag="ql_sl")
        nc.sync.dma_start(out=ql_sl, in_=ql_slab[:, :])
        ql_o = p_cl.tile([S, CA], F32, tag="ql_o")
        nc.vector.tensor_tensor(out=ql_o, in0=ql_ps, in1=ql_sl, op=ADD)
        nc.sync.dma_start(out=ql_out[:, :], in_=ql_o)

    nc.finalize()
    return nc


class TileCtx:
    """TileContext + ExitStack in one `with`."""

    def __init__(self, nc):
        self.nc = nc

    def __enter__(self):
        self.ctx = ExitStack()
        self.tc = self.ctx.enter_context(tile.TileContext(self.nc))
        return self.tc, self.ctx

    def __exit__(self, *exc):
        return self.ctx.__exit__(*exc)


def get_program():
    if "nc" not in _PROG_CACHE:
        _PROG_CACHE["nc"] = build_program()
    return _PROG_CACHE["nc"]


def _prep_inputs(inputs):
    f = lambda k: np.ascontiguousarray(np.asarray(inputs[k], dtype=np.float32))
    a2t = f("atom_to_token_index")
    cl = f("cl")
    plm = f("plm")
    ql = f("ql")
    s_trunk = f("s_trunk")
    zij = f("zij")
    rl = f("rl")
    ln_s_g = f("ln_s_g")
    ln_s_b = f("ln_s_b")
    W_s = f("W_s")
    ln_z_g = f("ln_z_g")
    ln_z_b = f("ln_z_b")
    W_z = f("W_z")
    W_r = f("W_r")

    tok = np.argmax(a2t, axis=1)
    perm = np.argsort(tok, kind="stable")

    wzp = np.ascontiguousarray(ln_z_g[:, None] * W_z)
    bz = np.ascontiguousarray((ln_z_b @ W_z)[:, None])          # [CP, 1]
    wsp = np.ascontiguousarray(ln_s_g[:, None] * W_s)
    bs = np.ascontiguousarray((ln_s_b @ W_s)[None, :])          # [1, CA]
    a2tT = np.ascontiguousarray(a2t.T)

    in_maps = []
    slabs = []
    for d in range(M):
        sl = perm[d * S : (d + 1) * S]
        ts_ = tok[sl]
        uts = np.unique(ts_)
        assert len(uts) <= W, f"{len(uts)} distinct tokens exceeds W={W}"
        idx = np.concatenate([uts, np.full(W - len(uts), uts[-1], uts.dtype)])
        pos = np.searchsorted(uts, ts_)
        a2tT_win = np.zeros((W, S), np.float32)
        a2tT_win[pos, np.arange(S)] = 1.0
        in_maps.append(
            {
                "zij_win": np.ascontiguousarray(zij[idx]),
                "a2tT": a2tT,
                "a2tT_win": a2tT_win,
                "plm_slab": np.ascontiguousarray(plm[sl]),
                "cl_slab": np.ascontiguousarray(cl[sl]),
                "ql_slab": np.ascontiguousarray(ql[sl]),
                "rlT_slab": np.ascontiguousarray(rl[sl].T),
                "strunk_win": np.ascontiguousarray(s_trunk[idx]),
                "wzp": wzp,
                "bz": bz,
                "wsp": wsp,
                "bs": bs,
                "wr": W_r,
            }
        )
        slabs.append(sl)
    return in_maps, slabs


def kernel(**inputs):
    in_maps, slabs = _prep_inputs(inputs)
    nc = get_program()
    res = run_bass_kernel_spmd(nc, in_maps, core_ids=list(range(M)))
    outs = res.results

    cl_full = np.empty((A, CA), np.float32)
    ql_full = np.empty((A, CA), np.float32)
    plm_full = np.empty((A, A, CP), np.float32)
    for d in range(M):
        sl = slabs[d]
        cl_full[sl] = outs[d]["cl_out"]
        ql_full[sl] = outs[d]["ql_out"]
        plm_full[sl] = outs[d]["plm_out"].reshape(S, A, CP)
    return (cl_full, plm_full, ql_full)
